# revision 16
# baseline (speedup 1.0000x reference)
"""Trainium2 Bass kernel for block-adapter Linear (nn_Linear_20847771255232).

Math:
    y = x @ W_base^T + b_base + s * adapter(x)
where the block-structured adapter folds into an effective weight:
    W_eff[o_blk*64+e, i*64+d] = W_base[...] + s * U[d, i, o_blk] * V[o_blk, d, e]
(no reduction in the adapter weight -- pure elementwise construction), so the
whole problem is ONE 4096x4096x4096 GEMM with an on-device-constructed weight.

Sharding (8 cores): 4-way data parallel over tokens (m) x 2-way tensor
parallel over out features (o).  Each core:
    x_s [1024, 4096] f32, w_s [2048, 4096] f32, b_s [2048], u_s [64, 64, 32],
    v_s [32, 64, 64], s_s [1]  ->  y_s [1024, 2048] f32

On-device pipeline per core:
  - load x tiles with SWDGE cast-DMA (f32->bf16), xbar-DMA-transpose into
    xT [128k, 32ki, 1024m] resident in SBUF
  - per 512-wide o-chunk: cast-load W rows, xbar-transpose to WT [128k, ki, 512o],
    add adapter in place (U broadcast via small PE matmul against a block-identity
    "E ones" matrix, multiply by V replicated over both 64-partition halves),
    then 8 m-tiles x 32 k-tiles of bf16 matmuls accumulating in PSUM
    (bias seeded via a K=1 matmul), drain with ScalarE copies, store via SWDGE.
"""

import numpy as np

BLOCK = 64
M_TOT, K_TOT, O_TOT = 4096, 4096, 4096
GRID_M, GRID_O = 4, 2
M_C, O_C = M_TOT // GRID_M, O_TOT // GRID_O  # 1024, 2048
O5 = 256

_CACHE = {}


def build_program(m_c=M_C, k=K_TOT, o_c=O_C, o5=O5, num_devices=8, debug=False):
    import concourse.bacc as bacc
    import concourse.bass as bass
    import concourse.mybir as mybir
    import concourse.tile as tile

    f32 = mybir.dt.float32
    bf16 = mybir.dt.bfloat16

    S = k // 128          # k-stripes of 128
    NB_I = k // BLOCK     # input blocks
    NB_O = o_c // BLOCK   # output blocks on this core (must be mult of 32)
    MT = m_c // 128
    NOC = o_c // o5
    WPC = o5 // 128
    assert NB_O % 32 == 0 and NB_O <= 128

    nc = bacc.Bacc(
        "TRN2", target_bir_lowering=False, debug=debug, num_devices=num_devices
    )
    x_d = nc.dram_tensor("x_s", [m_c, k], f32, kind="ExternalInput").ap()
    w_d = nc.dram_tensor("w_s", [o_c, k], f32, kind="ExternalInput").ap()
    b_d = nc.dram_tensor("b_s", [o_c], f32, kind="ExternalInput").ap()
    u_d = nc.dram_tensor("u_s", [BLOCK, NB_I, NB_O], f32, kind="ExternalInput").ap()
    v_d = nc.dram_tensor("v_s", [NB_O, BLOCK, BLOCK], f32, kind="ExternalInput").ap()
    s_d = nc.dram_tensor("s_s", [1], f32, kind="ExternalInput").ap()
    y_d = nc.dram_tensor("y_s", [m_c, o_c], f32, kind="ExternalOutput").ap()

    with tile.TileContext(nc) as tc:
        with (
            tc.tile_pool(name="const", bufs=1) as cpool,
            tc.tile_pool(name="xstage", bufs=2) as xstpool,
            tc.tile_pool(name="wstage", bufs=3) as wstpool,
            tc.tile_pool(name="xt", bufs=1) as xtpool,
            tc.tile_pool(name="wt", bufs=3) as wtpool,
            tc.tile_pool(name="adap", bufs=3) as apool,
            tc.tile_pool(name="outp", bufs=3) as opool,
            tc.tile_pool(name="psum_mm", bufs=4, space=bass.MemorySpace.PSUM) as pspool,
            tc.tile_pool(name="psum_ub", bufs=4, space=bass.MemorySpace.PSUM) as ubpool,
        ):
            # ---------------- phase 0: constants ----------------
            ones_f32 = cpool.tile([1, 128], f32)
            nc.vector.memset(ones_f32[:], 1.0)
            ones_bf = cpool.tile([1, 128], bf16)
            nc.vector.tensor_copy(ones_bf[:], ones_f32[:])

            b_bf = cpool.tile([1, o_c], bf16)
            nc.gpsimd.dma_start(b_bf[:], b_d[None, :])  # cast f32->bf16
            s_sb = cpool.tile([1, 1], f32)
            nc.gpsimd.dma_start(s_sb[:], s_d[None, :])
            u_bf = cpool.tile([BLOCK, NB_I * NB_O], bf16)
            nc.gpsimd.dma_start(u_bf[:], u_d.rearrange("d i o -> d (i o)"))
            v_rep = cpool.tile([128, o_c], bf16)
            nc.gpsimd.dma_start(
                v_rep[0:BLOCK, :].rearrange("d (j e) -> d j e", e=BLOCK),
                v_d.rearrange("j d e -> d j e"),
            )
            nc.gpsimd.dma_start(
                v_rep[BLOCK : 2 * BLOCK, :].rearrange("d (j e) -> d j e", e=BLOCK),
                v_d.rearrange("j d e -> d j e"),
            )

            # scale U by s = S[0] (SCALING == 1.0)
            s_ps = ubpool.tile([BLOCK, 1], f32, tag="ub")
            nc.tensor.matmul(
                s_ps[:], ones_f32[:, 0:BLOCK], s_sb[:], start=True, stop=True
            )
            s_col = cpool.tile([BLOCK, 1], f32)
            nc.vector.tensor_copy(s_col[:], s_ps[:])
            nc.vector.tensor_scalar_mul(u_bf[:], u_bf[:], s_col[:])

            # U_colsT[j, ki*128 + h*64 + d] = s * U[d, 2ki+h, j]   (bf16)
            ucolsT = cpool.tile([NB_O, S * 128], bf16)
            for ki in range(S):
                for h in range(2):
                    i = 2 * ki + h
                    for r in range(2):  # 32-row halves of d
                        for c in range(NB_O // 32):
                            nc.vector.transpose(
                                ucolsT[
                                    32 * c : 32 * c + 32,
                                    ki * 128 + h * 64 + 32 * r : ki * 128
                                    + h * 64
                                    + 32 * r
                                    + 32,
                                ],
                                u_bf[
                                    32 * r : 32 * r + 32,
                                    i * NB_O + 32 * c : i * NB_O + 32 * c + 32,
                                ],
                            )

            # E-ones: row j has ones on cols [j*64, (j+1)*64).
            # Built with two affine selects: keep 1.0 only where (j - jc) == 0.
            eones = cpool.tile([NB_O, o_c], bf16)
            nc.gpsimd.memset(eones[:], 1.0)
            eones3 = eones[:].rearrange("j (jc e) -> j jc e", e=BLOCK)
            nc.gpsimd.affine_select(
                out=eones3,
                in_=eones3,
                compare_op=mybir.AluOpType.is_ge,
                fill=0.0,
                base=0,
                pattern=[[-1, NB_O], [0, BLOCK]],
                channel_multiplier=1,
            )
            nc.gpsimd.affine_select(
                out=eones3,
                in_=eones3,
                compare_op=mybir.AluOpType.is_ge,
                fill=0.0,
                base=0,
                pattern=[[1, NB_O], [0, BLOCK]],
                channel_multiplier=-1,
            )

            # ---------------- phase 1/2: W chunk 0 first, then x stream ----
            def load_w_chunk(oc):
                wts = wtpool.tile([128, S, o5], bf16, tag="wt")
                for wc in range(WPC):
                    wbf = wstpool.tile([128, k], bf16, tag="wstage")
                    nc.gpsimd.dma_start(
                        wbf[:],
                        w_d[oc * o5 + wc * 128 : oc * o5 + (wc + 1) * 128, :],
                    )
                    nc.sync.dma_start_transpose(
                        wts[:, :, wc * 128 : (wc + 1) * 128], wbf[:]
                    )
                return wts

            # x tiles: cast-load on SWDGE, transpose on the ACT HWDGE queue
            # (W transposes use the SP queue — two parallel xbar streams).
            xT = xtpool.tile([128, S, m_c], bf16)

            def load_x_tile(mt):
                xbf = xstpool.tile([128, k], bf16, tag="xstage")
                nc.gpsimd.dma_start(xbf[:], x_d[mt * 128 : (mt + 1) * 128, :])
                nc.scalar.dma_start_transpose(
                    xT[:, :, mt * 128 : (mt + 1) * 128], xbf[:]
                )

            wts_q = [load_w_chunk(0)]
            for mt in range(min(2, MT)):
                load_x_tile(mt)
            wts_q.append(load_w_chunk(1) if NOC > 1 else None)
            for mt in range(2, MT):
                load_x_tile(mt)

            for oc in range(NOC):
                wts_cur = wts_q[0]
                wts_q = [wts_q[1], load_w_chunk(oc + 2) if oc + 2 < NOC else None]

                # adapter: wts[:, ki, :] += s*U[d,i,j]*V[j,d,e]
                # processed in ki-pairs so elementwise ops run at [128, 512]
                vb = (
                    v_rep[:, oc * o5 : (oc + 1) * o5]
                    .rearrange("p (one f) -> p one f", one=1)
                    .broadcast_to([128, 2, o5])
                )
                for kp in range(S // 2):
                    ub2 = ubpool.tile([128, 2, o5], f32, tag="ub")
                    for h2 in range(2):
                        ki = 2 * kp + h2
                        nc.tensor.matmul(
                            ub2[:, h2, :],
                            ucolsT[:, ki * 128 : (ki + 1) * 128],
                            eones[:, oc * o5 : (oc + 1) * o5],
                            start=True,
                            stop=True,
                        )
                    ad2 = apool.tile([128, 2, o5], bf16, tag="adap")
                    nc.vector.tensor_mul(ad2[:], ub2[:], vb)
                    nc.gpsimd.tensor_add(
                        wts_cur[:, 2 * kp : 2 * kp + 2, :],
                        ad2[:],
                        wts_cur[:, 2 * kp : 2 * kp + 2, :],
                    )

                # GEMM: 8 m-tiles, K-contiguous per tile
                for mt in range(MT):
                    ps = pspool.tile([128, o5], f32, tag="ps")
                    nc.tensor.matmul(
                        ps[:],
                        ones_bf[:],
                        b_bf[:, oc * o5 : (oc + 1) * o5],
                        start=True,
                        stop=False,
                    )
                    for ki in range(S):
                        nc.tensor.matmul(
                            ps[:],
                            xT[:, ki, mt * 128 : (mt + 1) * 128],
                            wts_cur[:, ki, :],
                            start=False,
                            stop=(ki == S - 1),
                        )
                    osb = opool.tile([128, o5], f32, tag="o")
                    nc.scalar.copy(osb[:], ps[:])
                    nc.gpsimd.dma_start(
                        y_d[mt * 128 : (mt + 1) * 128, oc * o5 : (oc + 1) * o5],
                        osb[:],
                    )

    nc.compile()
    return nc


def _get_program():
    key = "full"
    if key not in _CACHE:
        _CACHE[key] = build_program()
    return _CACHE[key]


def kernel(x, W_base, b_base, U, V, S):
    from concourse import bass_utils

    x = np.asarray(x, dtype=np.float32)
    W_base = np.asarray(W_base, dtype=np.float32)
    b_base = np.asarray(b_base, dtype=np.float32)
    U = np.asarray(U, dtype=np.float32)
    V = np.asarray(V, dtype=np.float32)
    S = np.asarray(S, dtype=np.float32)

    B, N, DIN = x.shape
    xf = np.ascontiguousarray(x.reshape(B * N, DIN))

    nc = _get_program()

    in_maps = []
    for c in range(8):
        mc, oc = divmod(c, GRID_O)
        in_maps.append(
            {
                "x_s": np.ascontiguousarray(xf[mc * M_C : (mc + 1) * M_C]),
                "w_s": np.ascontiguousarray(W_base[oc * O_C : (oc + 1) * O_C]),
                "b_s": np.ascontiguousarray(b_base[oc * O_C : (oc + 1) * O_C]),
                "u_s": np.ascontiguousarray(U[:, :, oc * (O_C // BLOCK) : (oc + 1) * (O_C // BLOCK)]),
                "v_s": np.ascontiguousarray(V[oc * (O_C // BLOCK) : (oc + 1) * (O_C // BLOCK)]),
                "s_s": S,
            }
        )

    res = bass_utils.run_bass_kernel_spmd(nc, in_maps, core_ids=list(range(8)))

    y = np.empty((B * N, O_TOT), dtype=np.float32)
    for c in range(8):
        mc, oc = divmod(c, GRID_O)
        y[mc * M_C : (mc + 1) * M_C, oc * O_C : (oc + 1) * O_C] = res.results[c]["y_s"]
    return y.reshape(B, N, O_TOT)


# revision 17
# speedup vs baseline: 1.0227x; 1.0227x over previous
"""Trainium2 Bass kernel for block-adapter Linear (nn_Linear_20847771255232).

Math:
    y = x @ W_base^T + b_base + s * adapter(x)
where the block-structured adapter folds into an effective weight:
    W_eff[o_blk*64+e, i*64+d] = W_base[...] + s * U[d, i, o_blk] * V[o_blk, d, e]
(no reduction in the adapter weight -- pure elementwise construction), so the
whole problem is ONE 4096x4096x4096 GEMM with an on-device-constructed weight.

Sharding (8 cores): 4-way data parallel over tokens (m) x 2-way tensor
parallel over out features (o).  Each core:
    x_s [1024, 4096] f32, w_s [2048, 4096] f32, b_s [2048], u_s [64, 64, 32],
    v_s [32, 64, 64], s_s [1]  ->  y_s [1024, 2048] f32

On-device pipeline per core:
  - load x tiles with SWDGE cast-DMA (f32->bf16), xbar-DMA-transpose into
    xT [128k, 32ki, 1024m] resident in SBUF
  - per 512-wide o-chunk: cast-load W rows, xbar-transpose to WT [128k, ki, 512o],
    add adapter in place (U broadcast via small PE matmul against a block-identity
    "E ones" matrix, multiply by V replicated over both 64-partition halves),
    then 8 m-tiles x 32 k-tiles of bf16 matmuls accumulating in PSUM
    (bias seeded via a K=1 matmul), drain with ScalarE copies, store via SWDGE.
"""

import numpy as np

BLOCK = 64
M_TOT, K_TOT, O_TOT = 4096, 4096, 4096
GRID_M, GRID_O = 4, 2
M_C, O_C = M_TOT // GRID_M, O_TOT // GRID_O  # 1024, 2048
O5 = 256

_CACHE = {}


def build_program(m_c=M_C, k=K_TOT, o_c=O_C, o5=O5, num_devices=8, debug=False):
    import concourse.bacc as bacc
    import concourse.bass as bass
    import concourse.mybir as mybir
    import concourse.tile as tile

    f32 = mybir.dt.float32
    bf16 = mybir.dt.bfloat16

    S = k // 128          # k-stripes of 128
    NB_I = k // BLOCK     # input blocks
    NB_O = o_c // BLOCK   # output blocks on this core (must be mult of 32)
    MT = m_c // 128
    NOC = o_c // o5
    WPC = o5 // 128
    assert NB_O % 32 == 0 and NB_O <= 128

    nc = bacc.Bacc(
        "TRN2", target_bir_lowering=False, debug=debug, num_devices=num_devices
    )
    x_d = nc.dram_tensor("x_s", [m_c, k], f32, kind="ExternalInput").ap()
    w_d = nc.dram_tensor("w_s", [o_c, k], f32, kind="ExternalInput").ap()
    b_d = nc.dram_tensor("b_s", [o_c], f32, kind="ExternalInput").ap()
    u_d = nc.dram_tensor("u_s", [BLOCK, NB_I, NB_O], f32, kind="ExternalInput").ap()
    v_d = nc.dram_tensor("v_s", [NB_O, BLOCK, BLOCK], f32, kind="ExternalInput").ap()
    s_d = nc.dram_tensor("s_s", [1], f32, kind="ExternalInput").ap()
    y_d = nc.dram_tensor("y_s", [m_c, o_c], f32, kind="ExternalOutput").ap()

    with tile.TileContext(nc) as tc:
        with (
            tc.tile_pool(name="const", bufs=1) as cpool,
            tc.tile_pool(name="xstage", bufs=2) as xstpool,
            tc.tile_pool(name="wstage", bufs=3) as wstpool,
            tc.tile_pool(name="xt", bufs=1) as xtpool,
            tc.tile_pool(name="wt", bufs=3) as wtpool,
            tc.tile_pool(name="adap", bufs=3) as apool,
            tc.tile_pool(name="outp", bufs=3) as opool,
            tc.tile_pool(name="psum_mm", bufs=4, space=bass.MemorySpace.PSUM) as pspool,
            tc.tile_pool(name="psum_ub", bufs=4, space=bass.MemorySpace.PSUM) as ubpool,
        ):
            # ---------------- phase 0: constants ----------------
            ones_f32 = cpool.tile([1, 128], f32)
            nc.vector.memset(ones_f32[:], 1.0)
            ones_bf = cpool.tile([1, 128], bf16)
            nc.vector.tensor_copy(ones_bf[:], ones_f32[:])

            b_bf = cpool.tile([1, o_c], bf16)
            nc.gpsimd.dma_start(b_bf[:], b_d[None, :])  # cast f32->bf16
            s_sb = cpool.tile([1, 1], f32)
            nc.gpsimd.dma_start(s_sb[:], s_d[None, :])
            u_bf = cpool.tile([BLOCK, NB_I * NB_O], bf16)
            nc.gpsimd.dma_start(u_bf[:], u_d.rearrange("d i o -> d (i o)"))
            v_rep = cpool.tile([128, o_c], bf16)
            nc.gpsimd.dma_start(
                v_rep[0:BLOCK, :].rearrange("d (j e) -> d j e", e=BLOCK),
                v_d.rearrange("j d e -> d j e"),
            )
            nc.gpsimd.dma_start(
                v_rep[BLOCK : 2 * BLOCK, :].rearrange("d (j e) -> d j e", e=BLOCK),
                v_d.rearrange("j d e -> d j e"),
            )

            # scale U by s = S[0] (SCALING == 1.0)
            s_ps = ubpool.tile([BLOCK, 1], f32, tag="ub")
            nc.tensor.matmul(
                s_ps[:], ones_f32[:, 0:BLOCK], s_sb[:], start=True, stop=True
            )
            s_col = cpool.tile([BLOCK, 1], f32)
            nc.vector.tensor_copy(s_col[:], s_ps[:])
            nc.vector.tensor_scalar_mul(u_bf[:], u_bf[:], s_col[:])

            # U_colsT[j, ki*128 + h*64 + d] = s * U[d, 2ki+h, j]   (bf16)
            ucolsT = cpool.tile([NB_O, S * 128], bf16)
            for ki in range(S):
                for h in range(2):
                    i = 2 * ki + h
                    for r in range(2):  # 32-row halves of d
                        for c in range(NB_O // 32):
                            nc.vector.transpose(
                                ucolsT[
                                    32 * c : 32 * c + 32,
                                    ki * 128 + h * 64 + 32 * r : ki * 128
                                    + h * 64
                                    + 32 * r
                                    + 32,
                                ],
                                u_bf[
                                    32 * r : 32 * r + 32,
                                    i * NB_O + 32 * c : i * NB_O + 32 * c + 32,
                                ],
                            )

            # E-ones: row j has ones on cols [j*64, (j+1)*64).
            # Built with two affine selects: keep 1.0 only where (j - jc) == 0.
            eones = cpool.tile([NB_O, o_c], bf16)
            nc.gpsimd.memset(eones[:], 1.0)
            eones3 = eones[:].rearrange("j (jc e) -> j jc e", e=BLOCK)
            nc.gpsimd.affine_select(
                out=eones3,
                in_=eones3,
                compare_op=mybir.AluOpType.is_ge,
                fill=0.0,
                base=0,
                pattern=[[-1, NB_O], [0, BLOCK]],
                channel_multiplier=1,
            )
            nc.gpsimd.affine_select(
                out=eones3,
                in_=eones3,
                compare_op=mybir.AluOpType.is_ge,
                fill=0.0,
                base=0,
                pattern=[[1, NB_O], [0, BLOCK]],
                channel_multiplier=-1,
            )

            # ---------------- phase 1/2: W chunk 0 first, then x stream ----
            def load_w_chunk(oc):
                wts = wtpool.tile([128, S, o5], bf16, tag="wt")
                for wc in range(WPC):
                    wbf = wstpool.tile([128, k], bf16, tag="wstage")
                    nc.gpsimd.dma_start(
                        wbf[:],
                        w_d[oc * o5 + wc * 128 : oc * o5 + (wc + 1) * 128, :],
                    )
                    nc.sync.dma_start_transpose(
                        wts[:, :, wc * 128 : (wc + 1) * 128], wbf[:]
                    )
                return wts

            # x tiles: cast-load on SWDGE, transpose on the ACT HWDGE queue
            # (W transposes use the SP queue — two parallel xbar streams).
            xT = xtpool.tile([128, S, m_c], bf16)

            def load_x_tile(mt):
                xbf = xstpool.tile([128, k], bf16, tag="xstage")
                nc.gpsimd.dma_start(xbf[:], x_d[mt * 128 : (mt + 1) * 128, :])
                nc.scalar.dma_start_transpose(
                    xT[:, :, mt * 128 : (mt + 1) * 128], xbf[:]
                )

            wts_q = [load_w_chunk(0)]
            for mt in range(min(2, MT)):
                load_x_tile(mt)
            wts_q.append(load_w_chunk(1) if NOC > 1 else None)
            for mt in range(2, MT):
                load_x_tile(mt)

            for oc in range(NOC):
                wts_cur = wts_q[0]
                wts_q = [wts_q[1], load_w_chunk(oc + 2) if oc + 2 < NOC else None]

                # adapter: wts[:, ki, :] += s*U[d,i,j]*V[j,d,e]
                # processed in ki-pairs so elementwise ops run at [128, 512]
                vb = (
                    v_rep[:, oc * o5 : (oc + 1) * o5]
                    .rearrange("p (one f) -> p one f", one=1)
                    .broadcast_to([128, 2, o5])
                )
                for kp in range(S // 2):
                    ub2 = ubpool.tile([128, 2, o5], f32, tag="ub")
                    for h2 in range(2):
                        ki = 2 * kp + h2
                        nc.tensor.matmul(
                            ub2[:, h2, :],
                            ucolsT[:, ki * 128 : (ki + 1) * 128],
                            eones[:, oc * o5 : (oc + 1) * o5],
                            start=True,
                            stop=True,
                        )
                    ad2 = apool.tile([128, 2, o5], bf16, tag="adap")
                    nc.vector.tensor_mul(ad2[:], ub2[:], vb)
                    nc.gpsimd.tensor_add(
                        wts_cur[:, 2 * kp : 2 * kp + 2, :],
                        ad2[:],
                        wts_cur[:, 2 * kp : 2 * kp + 2, :],
                    )

                # GEMM: 8 m-tiles, K-contiguous per tile
                for mt in range(MT):
                    ps = pspool.tile([128, o5], f32, tag="ps")
                    nc.tensor.matmul(
                        ps[:],
                        ones_bf[:],
                        b_bf[:, oc * o5 : (oc + 1) * o5],
                        start=True,
                        stop=False,
                    )
                    for ki in range(S):
                        nc.tensor.matmul(
                            ps[:],
                            xT[:, ki, mt * 128 : (mt + 1) * 128],
                            wts_cur[:, ki, :],
                            start=False,
                            stop=(ki == S - 1),
                        )
                    osb = opool.tile([128, o5], f32, tag="o")
                    nc.vector.tensor_copy(osb[:], ps[:])
                    nc.gpsimd.dma_start(
                        y_d[mt * 128 : (mt + 1) * 128, oc * o5 : (oc + 1) * o5],
                        osb[:],
                    )

    nc.compile()
    return nc


def _get_program():
    key = "full"
    if key not in _CACHE:
        _CACHE[key] = build_program()
    return _CACHE[key]


def kernel(x, W_base, b_base, U, V, S):
    from concourse import bass_utils

    x = np.asarray(x, dtype=np.float32)
    W_base = np.asarray(W_base, dtype=np.float32)
    b_base = np.asarray(b_base, dtype=np.float32)
    U = np.asarray(U, dtype=np.float32)
    V = np.asarray(V, dtype=np.float32)
    S = np.asarray(S, dtype=np.float32)

    B, N, DIN = x.shape
    xf = np.ascontiguousarray(x.reshape(B * N, DIN))

    nc = _get_program()

    in_maps = []
    for c in range(8):
        mc, oc = divmod(c, GRID_O)
        in_maps.append(
            {
                "x_s": np.ascontiguousarray(xf[mc * M_C : (mc + 1) * M_C]),
                "w_s": np.ascontiguousarray(W_base[oc * O_C : (oc + 1) * O_C]),
                "b_s": np.ascontiguousarray(b_base[oc * O_C : (oc + 1) * O_C]),
                "u_s": np.ascontiguousarray(U[:, :, oc * (O_C // BLOCK) : (oc + 1) * (O_C // BLOCK)]),
                "v_s": np.ascontiguousarray(V[oc * (O_C // BLOCK) : (oc + 1) * (O_C // BLOCK)]),
                "s_s": S,
            }
        )

    res = bass_utils.run_bass_kernel_spmd(nc, in_maps, core_ids=list(range(8)))

    y = np.empty((B * N, O_TOT), dtype=np.float32)
    for c in range(8):
        mc, oc = divmod(c, GRID_O)
        y[mc * M_C : (mc + 1) * M_C, oc * O_C : (oc + 1) * O_C] = res.results[c]["y_s"]
    return y.reshape(B, N, O_TOT)
